# revision 20
# baseline (speedup 1.0000x reference)
"""Trainium2 Bass kernel for nn_DiscriminatorLatent (dense MLP discriminator).

Strategy (pure data parallel over 8 NeuronCores):
 - batch dim (8192) sharded 1024 rows/core; weights replicated.
 - Feature-major layout: last^T kept in SBUF as 128x1024 bf16 chunks; layer
   matmuls compute z^T = (W^T).T @ last^T in bf16 with fp32 PSUM accumulation.
 - PSUM-resident accumulation: layer L's four m-chunk accumulators live in
   PSUM across two phases -- A-phase (chunks 0..4L-1, overlapping the stats
   exchange of layer L-1) and B-phase (the 4 newest chunks, right after the
   retire).  zpool bufs=4 covers all 8 PSUM banks.
 - BatchNorm batch stats: per-core (mean, E[x^2]) via one DVE
   bn_stats+bn_aggr pass over the fp32 PSUM quad; cross-core reduction is a
   hand-rolled XOR all-gather over SWDGE remote DMA broadcasts (each core
   broadcasts its [128,4,2] stats tile to peer slot k via rdests=(0,k); the
   receiver's slot k holds sender my_id^k, and the slot-sum is
   order-invariant).  ~3us instead of the ~30us collectives-stack AllReduce
   (and no ~60us CC-stack init at kernel start).  Sems: arrivals bump rsem
   (+2/broadcast, 14/round); the layer-i reduce waits rsem >= 14(i+1);
   rsem is cleared at kernel end (after the last wait) so repeated NEFF
   executions start from zero.  Recv slots are double-buffered by round
   parity; the retire chain bounds sender skew to < 2 rounds.
 - rstd computed on DVE with a bitcast Newton rsqrt (no ACT Sqrt -> the ACT
   activation table stays on LeakyReLU; no per-layer table reloads).
 - PSUM drains to bf16 lastT chunks on ACT (Copy); LeakyReLU+BN affine on
   ACT; noise multiply on DVE (noise pre-cast to bf16 host-side).
 - The linear bias b cancels inside training-mode BN and is not applied.
 - Logits (Wc . last): 56 chunk matmuls during the final exchange window;
   the last layer's 8 follow its retire.  Sigmoid on ACT.
"""

import os
import sys

if "/opt/trn_rl_repo" not in sys.path:
    sys.path.insert(0, "/opt/trn_rl_repo")

import ml_dtypes
import numpy as np

import concourse.bass as bass
import concourse.tile as tile
from concourse import bacc, mybir
from concourse import bass_utils

F32 = mybir.dt.float32
U32 = mybir.dt.uint32
BF16 = mybir.dt.bfloat16
AF = mybir.ActivationFunctionType
ALU = mybir.AluOpType

N_CORES = 8
B = 8192
B_LOC = B // N_CORES  # 1024
LVS = 512
WIDTH = 512
DEPTH = 7
EPS = 1e-5
SLOPE = 0.01
NH = B_LOC // 512  # 512-wide halves per chunk (PSUM bank = 512 fp32)

MM_DT = BF16
MM_NP = ml_dtypes.bfloat16

TRACE = False
LAST_EXEC_NS = None
LAST_RESULTS = None
LAST_IN_MAPS = None

_BUILD_CACHE = {}

# rsem increments per arrived stats broadcast (16 lanes / 8 dest slots)
RSEM_PER_BCAST = 2
RSEM_PER_ROUND = RSEM_PER_BCAST * (N_CORES - 1)


def _build(depth=DEPTH):
    nc = bacc.Bacc("TRN2", target_bir_lowering=False, debug=False,
                   num_devices=N_CORES)

    n_chunks = 4 * (depth + 1)

    # ---- DRAM I/O ----------------------------------------------------------
    xt_d = nc.dram_tensor("xt", [LVS, B_LOC], MM_DT, kind="ExternalInput").ap()
    wt_d = [
        nc.dram_tensor(f"wt{i}", [LVS + WIDTH * i, WIDTH], MM_DT,
                       kind="ExternalInput").ap()
        for i in range(depth)
    ]
    noiset_d = nc.dram_tensor("noiset", [depth, WIDTH, B_LOC], MM_DT,
                              kind="ExternalInput").ap()
    wct_d = nc.dram_tensor("wct", [128, n_chunks], MM_DT,
                           kind="ExternalInput").ap()
    gammat_d = nc.dram_tensor("gammat", [128, 4 * depth], F32,
                              kind="ExternalInput").ap()
    betat_d = nc.dram_tensor("betat", [128, 4 * depth], F32,
                             kind="ExternalInput").ap()
    bct_d = nc.dram_tensor("bct", [1, 1], F32, kind="ExternalInput").ap()
    out_d = nc.dram_tensor("out", [1, B_LOC], F32, kind="ExternalOutput").ap()

    # ---- persistent SBUF ---------------------------------------------------
    lastT = [
        nc.alloc_sbuf_tensor(f"lastT{k}", [128, B_LOC], MM_DT).ap()
        for k in range(n_chunks)
    ]
    wct_sb = nc.alloc_sbuf_tensor("wct_sb", [128, n_chunks], MM_DT).ap()
    gammat_sb = nc.alloc_sbuf_tensor("gammat_sb", [128, 4 * depth], F32).ap()
    betat_sb = nc.alloc_sbuf_tensor("betat_sb", [128, 4 * depth], F32).ap()
    bct_sb = nc.alloc_sbuf_tensor("bct_sb", [1, 1], F32).ap()
    logits_acc = nc.alloc_sbuf_tensor("logits_acc", [1, B_LOC], F32).ap()
    out_sb = nc.alloc_sbuf_tensor("out_sb", [1, B_LOC], F32).ap()
    # stats recv slots: [round parity, sender slot 1..7, (m, mean|ex2)]
    recv_sb = nc.alloc_sbuf_tensor("recv_sb", [128, 2, N_CORES - 1, 8],
                                   F32).ap()

    # one arrival sem per layer/round, cleared after use so repeated NEFF
    # executions start from zero (clearing is race-free: the round's gate
    # waited for all 14 increments and nothing else targets that sem)
    rsems = [nc.alloc_semaphore(f"stats_rsem{i}") for i in range(depth)]
    lsem = nc.alloc_semaphore("stats_lsem")

    with tile.TileContext(nc) as tc:
        with (
            tc.tile_pool(name="wpool", bufs=34) as wpool,
            tc.tile_pool(name="npool", bufs=6) as npool,
            tc.tile_pool(name="stpool", bufs=6) as stpool,
            tc.tile_pool(name="fpool", bufs=24) as fpool,
            tc.tile_pool(name="zpool", bufs=4, space="PSUM") as zpool,
        ):
            # ---- preamble: Lrelu table preload + constant/x loads ----
            # memset on DVE, not gpsimd: the Pool engine then runs only
            # remote_dma-library instructions (no mid-kernel Q7 lib reload)
            eps_t = nc.alloc_sbuf_tensor("const_eps", [128, 1], F32)
            nc.vector.memset(eps_t.ap(), EPS)
            nc.const_aps.aps[(F32, EPS)] = eps_t.ap()
            dum = nc.alloc_sbuf_tensor("dum", [128, 1], F32)
            nc.scalar.activation(dum.ap()[:], eps_t.ap()[:], AF.Lrelu,
                                 bias=0.0, scale=1.0, alpha=SLOPE)
            for k in range(4):
                nc.sync.dma_start(lastT[k][:], xt_d[k * 128:(k + 1) * 128, :])
            nc.sync.dma_start(wct_sb[:], wct_d[:])
            nc.sync.dma_start(gammat_sb[:], gammat_d[:])
            nc.sync.dma_start(betat_sb[:], betat_d[:])
            nc.sync.dma_start(bct_sb[:], bct_d[:])

            def load_wblocks(i, ks):
                tiles = {}
                for k in ks:
                    wt = wpool.tile([128, WIDTH], MM_DT)
                    nc.sync.dma_start(wt[:], wt_d[i][k * 128:(k + 1) * 128, :])
                    tiles[k] = wt
                return tiles

            def mm_chunk(quads, wtiles, k, first, last):
                """k-major: one weight block feeds all 4 m accumulators."""
                for m in range(4):
                    for h in range(NH):
                        nc.tensor.matmul(
                            quads[m][:, h, :],
                            wtiles[k][:, m * 128:(m + 1) * 128],
                            lastT[k][:, h * 512:(h + 1) * 512],
                            start=first,
                            stop=last,
                        )

            def rsqrt_dve(y4, v4):
                """y4 = 1/sqrt(v4) via bitcast seed + 1 Newton step (DVE)."""
                vu = v4[:].bitcast(U32)
                yu = y4[:].bitcast(U32)
                nc.vector.tensor_scalar(
                    yu, vu, 1, None, op0=ALU.logical_shift_right)
                # DVE arithmetic on u32 APs is value-converted through fp32,
                # so compute MAGIC - (u>>1) as float math (no 2^32 wrap);
                # the <=64-ulp rounding noise is absorbed by the Newton step
                nc.vector.tensor_scalar(
                    yu, yu, -1.0, float(0x5F3759DF),
                    op0=ALU.mult, op1=ALU.add)
                # one Newton step leaves ~1.7e-3 relative error on rstd --
                # well inside the BN tolerance
                a4 = fpool.tile([128, 4], F32)
                nc.vector.tensor_mul(a4[:], y4[:], y4[:])
                nc.vector.tensor_mul(a4[:], a4[:], v4[:])
                nc.vector.tensor_scalar(
                    a4[:], a4[:], -0.5, 1.5, op0=ALU.mult, op1=ALU.add)
                nc.vector.tensor_mul(y4[:], y4[:], a4[:])

            def emit_finalize(i, gstats):
                """gstats [128,4,2] = (sum of core means | sum of core E[x^2])
                per m-chunk -> per-feature scale/shift for layer i's output."""
                nc.vector.tensor_scalar_mul(
                    gstats[:, :, :], gstats[:, :, :], 1.0 / N_CORES)
                mean4 = gstats[:, :, 0]
                ex24 = gstats[:, :, 1]
                v4 = fpool.tile([128, 4], F32)
                msq4 = fpool.tile([128, 4], F32)
                nc.vector.tensor_mul(msq4[:], mean4, mean4)
                nc.vector.tensor_sub(v4[:], ex24, msq4[:])
                nc.vector.tensor_scalar_add(v4[:], v4[:], EPS)
                y4 = fpool.tile([128, 4], F32)
                rsqrt_dve(y4, v4)
                scale4 = fpool.tile([128, 4], F32)
                shift4 = fpool.tile([128, 4], F32)
                nc.vector.tensor_mul(
                    scale4[:], y4[:], gammat_sb[:, 4 * i:4 * i + 4])
                nc.vector.scalar_tensor_tensor(
                    shift4[:], mean4, -1.0, scale4[:],
                    op0=ALU.mult, op1=ALU.mult)
                nc.vector.tensor_add(
                    shift4[:], shift4[:], betat_sb[:, 4 * i:4 * i + 4])
                return scale4, shift4

            def emit_norm(i, scale4, shift4, ntiles):
                """BN affine + LeakyReLU (ACT) then noise multiply (DVE),
                in place on layer i's output chunks."""
                for m in range(4):
                    ch = lastT[4 * (i + 1) + m]
                    nc.scalar.activation(
                        ch[:], ch[:], AF.Lrelu,
                        bias=shift4[:, m:m + 1],
                        scale=scale4[:, m:m + 1],
                        alpha=SLOPE,
                    )
                    nc.vector.tensor_mul(ch[:], ch[:], ntiles[m][:])

            def emit_drain_stats_m(i, quads, lstats, m):
                """Drain quad m PSUM->bf16 lastT (ACT) and one-pass local
                stats (DVE bn_stats+bn_aggr) -> lstats[:,m,:]=(mean,var)."""
                flat = quads[m][:, :, :]
                nc.scalar.copy(lastT[4 * (i + 1) + m][:], flat)
                st = fpool.tile([128, 2, 6], F32, tag="st", name=f"st_{i}_{m}")
                for h in range(NH):
                    nc.vector.bn_stats(st[:, h, :], quads[m][:, h, :])
                nc.vector.bn_aggr(lstats[:, m, :], st[:, :, :])

            def emit_stats_pack(lstats):
                """Convert var -> E[x^2] in place across all four m-chunks."""
                msq = fpool.tile([128, 4], F32)
                nc.vector.tensor_mul(msq[:], lstats[:, :, 0], lstats[:, :, 0])
                nc.vector.tensor_add(lstats[:, :, 1], lstats[:, :, 1], msq[:])

            def emit_exchange(i, lstats):
                """XOR all-gather of this core's stats tile: broadcast k
                writes peer (my_id ^ k)'s recv slot k-1 for this round's
                parity.  Descs are SWDGE prepare-only; the trigger fires once
                the stats tile is ready (Tile defers the RAW edge there)."""
                par = i % 2
                for k in range(1, N_CORES):
                    rdests = [None] * 8
                    rdests[k] = (0, k)
                    nc.gpsimd.remote_dma_broadcast(
                        recv_sb[:, par, k - 1, :],
                        lstats[:, :, :],
                        remote_sem=rsems[i],
                        local_sem=lsem,
                        rdests=rdests,
                    )
                nc.gpsimd.trigger_dma(count=None)

            def emit_reduce(i, lstats):
                """Sum own stats tile with the 7 peer tiles of round i.

                The arrival wait (rsem >= 14*(i+1)) cannot be emitted here:
                Tile's single-core scheduling sim would deadlock on it (no
                peers there).  Instead the first add is captured and the sem
                wait is attached post-scheduling (see deferred_waits)."""
                par = i % 2
                gsum = stpool.tile([128, 4, 2], F32)
                # data-anchored gate: copies lstats (so the scheduler keeps
                # it after the pack and before the adds, which read the copy)
                lst_g = stpool.tile([128, 4, 2], F32)
                gate = nc.vector.tensor_copy(lst_g[:, :, :], lstats[:, :, :])
                deferred_waits.append((gate, rsems[i], RSEM_PER_ROUND))
                nc.vector.tensor_tensor(
                    gsum[:, :, :], lst_g[:, :, :], recv_sb[:, par, 0, :],
                    op=ALU.add)
                for k in range(2, N_CORES):
                    nc.vector.tensor_tensor(
                        gsum[:, :, :], gsum[:, :, :],
                        recv_sb[:, par, k - 1, :], op=ALU.add)
                return gsum

            # ---- layer pipeline ----
            deferred_waits = []  # (gate BassInstruction, sem, threshold)
            pending = None      # (layer, lstats) exchange in flight
            quads = None        # current layer's PSUM accumulators
            noise_pending = {}  # layer -> 4 noise tiles
            for i in range(depth):
                new0 = 4 * i  # first B-phase chunk index of layer i

                if i == 0:
                    quads = [zpool.tile([128, NH, 512], F32, tag="z",
                                        name=f"qz0_{m}") for m in range(4)]
                    first_k = True
                else:
                    # retire layer i-1: reduce stats, finalize, norm
                    pi, lst = pending
                    gsum = emit_reduce(pi, lst)
                    scale4, shift4 = emit_finalize(pi, gsum)
                    emit_norm(pi, scale4, shift4, noise_pending.pop(pi))
                    pending = None
                    first_k = False

                # B-phase: 4 newest chunks, continuing the PSUM groups.
                # First 3 chunks k-major (pipelines behind the per-chunk
                # norm); the last chunk m-major with stop per quad, so each
                # quad's drain + bn_stats launches while the PE finishes the
                # remaining quads.
                wtiles_b = load_wblocks(i, range(new0, new0 + 4))
                lstats = stpool.tile([128, 4, 2], F32)
                for idx, k in enumerate(range(new0, new0 + 3)):
                    mm_chunk(quads, wtiles_b, k,
                             first=(first_k and idx == 0), last=False)
                k_last = new0 + 3
                for m in range(4):
                    for h in range(NH):
                        nc.tensor.matmul(
                            quads[m][:, h, :],
                            wtiles_b[k_last][:, m * 128:(m + 1) * 128],
                            lastT[k_last][:, h * 512:(h + 1) * 512],
                            start=False,
                            stop=True,
                        )
                    emit_drain_stats_m(i, quads, lstats, m)
                emit_stats_pack(lstats)
                emit_exchange(i, lstats)
                pending = (i, lstats)

                # A-phase of layer i+1: all chunks 0..4(i+1)-1 are ready
                if i + 1 < depth:
                    quads = [zpool.tile([128, NH, 512], F32, tag="z",
                                        name=f"qz{i + 1}_{m}")
                             for m in range(4)]
                    a_ks = range(4 * (i + 1))
                    wtiles_a = load_wblocks(i + 1, a_ks)
                    for idx, k in enumerate(a_ks):
                        mm_chunk(quads, wtiles_a, k,
                                 first=(idx == 0), last=False)

                # prefetch noise for layer i's retire (bf16)
                nts = []
                for m in range(4):
                    ntile = npool.tile([128, B_LOC], MM_DT)
                    nc.sync.dma_start(
                        ntile[:],
                        noiset_d[i:i + 1, m * 128:(m + 1) * 128, :])
                    nts.append(ntile)
                noise_pending[i] = nts

            # ---- tail ----
            # logits for all finished chunks (x + layers 0..depth-2) run
            # during the final exchange's flight
            lt1 = zpool.tile([1, NH, 512], F32, tag="z")
            done_chunks = 4 * depth  # chunks 0..4*depth-1 are normed
            for j in range(done_chunks):
                for h in range(NH):
                    nc.tensor.matmul(
                        lt1[:, h, :],
                        wct_sb[:, j:j + 1],
                        lastT[j][:, h * 512:(h + 1) * 512],
                        start=(j == 0),
                        stop=(j == done_chunks - 1),
                    )
            nc.vector.tensor_copy(logits_acc[:], lt1[:, :, :])

            # retire the last layer
            pi, lst = pending
            gsum = emit_reduce(pi, lst)
            scale4, shift4 = emit_finalize(pi, gsum)
            emit_norm(pi, scale4, shift4, noise_pending.pop(pi))

            lt2 = zpool.tile([1, NH, 512], F32, tag="z")
            for jj in range(4):
                j = done_chunks + jj
                for h in range(NH):
                    nc.tensor.matmul(
                        lt2[:, h, :],
                        wct_sb[:, j:j + 1],
                        lastT[j][:, h * 512:(h + 1) * 512],
                        start=(jj == 0),
                        stop=(jj == 3),
                    )
            nc.vector.tensor_tensor(
                logits_acc[:], logits_acc[:], lt2[:, :, :], op=ALU.add)

            # sigmoid(logits + bc) -> out
            nc.scalar.activation(
                out_sb[:], logits_acc[:], AF.Sigmoid, bias=bct_sb[:, :])
            nc.sync.dma_start(out_d[:], out_sb[:])



    # Splice the arrival waits in post-scheduling: Tile's single-core
    # scheduling sim would deadlock on a cross-core sem wait (and every
    # scheduled instruction's wait slot is already taken by the engine-chain
    # sem), so emit standalone DVE EventSemaphore waits now and move each
    # one directly before its data-anchored gate copy, with the sem clear
    # right after (all arrivals are in once the wait passes).
    fn = nc.m.functions[0]
    for inst, sem, thr in deferred_waits:
        wi = nc.vector.wait_ge(sem, thr).ins
        ci = nc.vector.sem_clear(sem).ins
        for x in (wi, ci):
            src = next(b for b in fn.blocks
                       if any(y.name == x.name for y in b.instructions))
            src.instructions = [y for y in src.instructions
                                if y.name != x.name]
        dst = next(b for b in fn.blocks
                   if any(x.name == inst.ins.name for x in b.instructions))
        lst = list(dst.instructions)
        at = [x.name for x in lst].index(inst.ins.name)
        lst.insert(at, wi)
        lst.insert(at + 2, ci)  # wait, gate copy, clear
        dst.instructions = lst

    nc.compile()
    return nc


def _get_nc(depth=DEPTH):
    if depth not in _BUILD_CACHE:
        _BUILD_CACHE[depth] = _build(depth)
    return _BUILD_CACHE[depth]


def _prep_core_inputs(c, depth, x, Ws, gamma, beta, Wc, bc, noise):
    n_chunks = 4 * (depth + 1)
    s = slice(c * B_LOC, (c + 1) * B_LOC)
    m = {}
    m["xt"] = np.ascontiguousarray(x[s].T).astype(MM_NP)
    for i in range(depth):
        m[f"wt{i}"] = np.ascontiguousarray(Ws[i].T).astype(MM_NP)
    m["noiset"] = np.ascontiguousarray(
        noise[:depth, s].transpose(0, 2, 1)).astype(MM_NP)
    wc_used = Wc[0, :128 * n_chunks]
    m["wct"] = np.ascontiguousarray(
        wc_used.reshape(n_chunks, 128).T).astype(MM_NP)
    m["gammat"] = np.ascontiguousarray(gamma[:depth].reshape(depth * 4, 128).T)
    m["betat"] = np.ascontiguousarray(beta[:depth].reshape(depth * 4, 128).T)
    m["bct"] = np.asarray(bc, dtype=np.float32).reshape(1, 1)
    return m


def _run(depth, x, Ws, gamma, beta, Wc, bc, noise):
    global LAST_EXEC_NS, LAST_RESULTS, LAST_IN_MAPS
    nc = _get_nc(depth)
    base = _prep_core_inputs(0, depth, x, Ws, gamma, beta, Wc, bc, noise)
    in_maps = [base]
    for c in range(1, N_CORES):
        m = dict(base)
        s = slice(c * B_LOC, (c + 1) * B_LOC)
        m["xt"] = np.ascontiguousarray(x[s].T).astype(MM_NP)
        m["noiset"] = np.ascontiguousarray(
            noise[:depth, s].transpose(0, 2, 1)).astype(MM_NP)
        in_maps.append(m)
    LAST_IN_MAPS = in_maps
    kwargs = {}
    if TRACE:
        kwargs["trace"] = True
    res = bass_utils.run_bass_kernel_spmd(
        nc, in_maps, core_ids=list(range(N_CORES)), **kwargs)
    LAST_EXEC_NS = res.exec_time_ns
    LAST_RESULTS = res
    out = np.empty((B, 1), dtype=np.float32)
    for c in range(N_CORES):
        out[c * B_LOC:(c + 1) * B_LOC, 0] = res.results[c]["out"][0]
    return out


def kernel(x, W0, W1, W2, W3, W4, W5, W6, b, gamma, beta, Wc, bc, noise):
    Ws = (W0, W1, W2, W3, W4, W5, W6)
    # the linear bias b cancels exactly inside training-mode BatchNorm
    return _run(DEPTH, np.asarray(x, np.float32),
                [np.asarray(w, np.float32) for w in Ws],
                np.asarray(gamma, np.float32), np.asarray(beta, np.float32),
                np.asarray(Wc, np.float32), np.asarray(bc, np.float32),
                np.asarray(noise, np.float32))
